# revision 18
# baseline (speedup 1.0000x reference)
"""Causal self-attention Trainium2 kernel.

B=2, T=2048, C=1024, H=16, D=64, 8 NeuronCores.
Sharding: core i handles batch b=i//4 and heads [4*(i%4), 4*(i%4)+4).
Host transposes x[b] -> xT, slices w_qkv/w_proj per core, and sums the 4
per-batch partial output projections at the end.

Dtypes: qkv path in bf16 (q,k,v errors ~0.4%/sqrt(fan-in); S matmuls and
AV matmuls run 1 cyc/row with FWL weight loads), scores/softmax in f32
PSUM -> bf16 P, output projection in fp8e4m3 with DoubleRow (0.5 cyc/row).

Scores are computed transposed (S^T[j,i]) so softmax exp/mask are free-dim
ops and P^T feeds the attention*V matmul as the moving operand. A ones
column appended to V yields the softmax denominator for free.

Head PAIRING: heads 2p/2p+1 share qT[p]/kTp[p] tiles (rows 0:64 / 64:128).
S^T for the pair runs as two concurrent K=64 matmuls on disjoint PE
row-groups into two adjacent PSUM banks; one ACT exp instruction covers
both banks.  Phase A (qkv projection) chunks interleave with attention
blocks in issue order, and each attention block's AV chain drains during
the NEXT block's S/exp steps, so the PE and ACT stay concurrently busy.
Softmax reciprocal runs on the DVE (reciprocal_approx_fast).
"""

import numpy as np
from contextlib import ExitStack

import concourse.bacc as bacc
import concourse.mybir as mybir
import concourse.tile as tile
from concourse.bass_utils import run_bass_kernel_spmd

B, T, C = 2, 2048, 1024
N_HEAD = 16
D = C // N_HEAD  # 64
N_CORES = 8
HPC = 4  # heads per core
NPAIR = 2  # head pairs per core
TB = T // 512  # 4 i-blocks of 512
NJ = T // 128  # 16 j-chunks of 128
VW = 128  # v chunk stride: 64 v-dims + ones columns (denominator at col 64)

F32 = mybir.dt.float32
F32R = mybir.dt.float32r
BF16 = mybir.dt.bfloat16
F8 = mybir.dt.float8e4

_compiled = None


def _build_mask():
    """M[j, c] = 1.0 iff c >= j, shape [128, 128] (triangular block mask)."""
    j = np.arange(128)[:, None]
    c = np.arange(128)[None, :]
    return (c >= j).astype(np.float32)


def _build_sel():
    """sel2[h, 128p + q]: pair-p broadcast selector.

    prf = sel2[:, 128p:128(p+1)].T @ recs gives prf rows 0:64 = recs[2p],
    rows 64:128 = recs[2p+1]."""
    sel = np.zeros((4, 256), dtype=np.float32)
    for p in range(2):
        sel[2 * p, 128 * p:128 * p + 64] = 1.0
        sel[2 * p + 1, 128 * p + 64:128 * (p + 1)] = 1.0
    return sel


def _build_nc():
    nc = bacc.Bacc("TRN2", target_bir_lowering=False, debug=False,
                   num_devices=N_CORES)

    xt_t = nc.dram_tensor("xt", [C, T], BF16, kind="ExternalInput")
    wqk_t = nc.dram_tensor("wqk", [C, 8 * D], BF16, kind="ExternalInput")
    wv_t = nc.dram_tensor("wv", [C, 4 * D], BF16, kind="ExternalInput")
    wp_t = nc.dram_tensor("wp", [128, 2 * C], BF16, kind="ExternalInput")
    mask_t = nc.dram_tensor("mask", [128, 128], BF16, kind="ExternalInput")
    sel_t = nc.dram_tensor("sel", [4, 256], F32R, kind="ExternalInput")
    vinit_t = nc.dram_tensor("vinit", [128, 64 * NJ], BF16,
                             kind="ExternalInput")
    out_t = nc.dram_tensor("out", [T, C], BF16, kind="ExternalOutput")

    Exp = mybir.ActivationFunctionType.Exp
    DR = mybir.MatmulPerfMode.DoubleRow

    with tile.TileContext(nc) as tc, ExitStack() as ctx:
        sb = ctx.enter_context(tc.tile_pool(name="sb", bufs=1))

        # ---- persistent SBUF (pool sb) ----
        mask_s = sb.tile([128, 128], BF16, tag="mask")
        sel_s = sb.tile([4, 256], F32R, tag="sel")
        # wp8 [128 d, 2 pair, 1024 c]: wp[128p + d, c]
        wp_s = sb.tile([128, 2 * C], BF16, tag="wp8")
        wpv = wp_s[:].rearrange("p (c x) -> p c x", c=2)
        qT = [sb.tile([128, T], BF16, tag=f"qT{p}", name=f"qT{p}")
              for p in range(2)]
        # kTp[p]: rows 0:64 = head 2p k, rows 64:128 = head 2p+1 k
        kTp = [sb.tile([128, T], BF16, tag=f"kT{p}", name=f"kT{p}")
               for p in range(NPAIR)]
        # vall viewed [128 t-part, 4 h, NJ chunk, VW]: cols 0:64 = v, col 64
        # = ones (softmax denominator via the AV matmul)
        vall = sb.tile([128, HPC * NJ * VW], BF16, tag="vall")
        vv = vall[:].rearrange("p (h c x) -> p h c x", h=HPC, x=VW)
        # yT8 [128 d, 2 pair, 2048 t]: normalized y, fp8 for DoubleRow proj
        yT8 = sb.tile([128, 2 * T], BF16, tag="yT8")
        yv = yT8[:].rearrange("p (c x) -> p c x", c=2)

        pa = ctx.enter_context(tc.tile_pool(name="pa", bufs=1))
        psa = ctx.enter_context(
            tc.tile_pool(name="psa", bufs=2, space="PSUM"))
        pb = ctx.enter_context(tc.tile_pool(name="pb", bufs=1))

        # ---- input DMAs (phase-B constants pushed behind via wait_until) --
        wqk_s = [pa.tile([128, 8 * D], BF16, tag=f"wqk{kc}",
                         name=f"wqk{kc}") for kc in range(8)]
        xt_s = {}
        for kc in range(8):
            nc.sync.dma_start(wqk_s[kc][:],
                              wqk_t.ap()[128 * kc:128 * (kc + 1), :])
            t = pa.tile([128, 512], BF16, tag=f"xt{kc}_0", name=f"xt{kc}_0")
            nc.sync.dma_start(t[:], xt_t.ap()[128 * kc:128 * (kc + 1),
                                              0:512])
            xt_s[(kc, 0)] = t
        wv_s = [pa.tile([128, 4 * D], BF16, tag=f"wv{kc}",
                        name=f"wv{kc}") for kc in range(8)]
        for kc in range(8):
            nc.sync.dma_start(wv_s[kc][:],
                              wv_t.ap()[128 * kc:128 * (kc + 1), :])
        with tc.tile_wait_until(0.004):
            nc.scalar.dma_start(mask_s[:], mask_t.ap()[:])
            nc.scalar.dma_start(sel_s[:], sel_t.ap()[:])
            for h in range(HPC):
                nc.scalar.dma_start(
                    vv[:, h, :, 64:128],
                    vinit_t.ap()[:].rearrange("p (c w) -> p c w", w=64))
        for nb in range(1, TB):
            with tc.tile_wait_until(0.0025 * nb):
                for kc in range(8):
                    t = pa.tile([128, 512], BF16, tag=f"xt{kc}_{nb}",
                                name=f"xt{kc}_{nb}")
                    nc.sync.dma_start(
                        t[:], xt_t.ap()[128 * kc:128 * (kc + 1),
                                        512 * nb:512 * (nb + 1)])
                    xt_s[(kc, nb)] = t
        with tc.tile_wait_until(0.012):
            nc.scalar.dma_start(wp_s[:], wp_t.ap()[:])

        def qk_chain(nb, mc):
            p = psa.tile([128, 512], F32, tag="mm")
            for kc in range(8):
                nc.tensor.matmul(
                    p[:], wqk_s[kc][:, 128 * mc:128 * (mc + 1)],
                    xt_s[(kc, nb)][:],
                    start=(kc == 0), stop=(kc == 7))
            dst = qT[mc] if mc < 2 else kTp[mc - 2]
            nc.vector.tensor_copy(dst[:, 512 * nb:512 * (nb + 1)], p[:])

        def v_chain(nb, tq):
            tci = 4 * nb + tq
            p = psa.tile([128, 256], F32, tag="mm")
            for kc in range(8):
                nc.tensor.matmul(
                    p[:], xt_s[(kc, nb)][:, 128 * tq:128 * (tq + 1)],
                    wv_s[kc][:], start=(kc == 0), stop=(kc == 7))
            nc.vector.tensor_copy(
                vv[:, :, tci, 0:64],
                p[:].rearrange("p (h d) -> p h d", h=HPC))

        def phase_a_chains(nb):
            # qT/kTp: mc 0,1 -> q pairs; 2,3 -> k pairs
            return ([(lambda nb=nb, mc=mc: qk_chain(nb, mc))
                     for mc in range(4)]
                    + [(lambda nb=nb, tq=tq: v_chain(nb, tq))
                       for tq in range(4)])

        # ---- phase B machinery ----
        pss = ctx.enter_context(
            tc.tile_pool(name="pss", bufs=2, space="PSUM"))
        psy = ctx.enter_context(
            tc.tile_pool(name="psy", bufs=1, space="PSUM"))

        drows = [pb.tile([4, 512], F32, tag=f"dr{ib}", name=f"dr{ib}")
                 for ib in range(TB)]
        # unnormalized y staging: slot (s head-in-pair, 4*p + ib)
        ytmp = pb.tile([64, 2 * 4096], F32R, tag="ytmp")

        def make_normproj(ib):
            def fin():
                recs_f = pb.tile([4, 512], F32, tag="recsf", bufs=2,
                                 name="recs_f")
                nc.vector.reciprocal_approx_fast(recs_f[:], drows[ib][:])
                recs = pb.tile([4, 512], F32R, tag="recs", bufs=2,
                               name="recs")
                nc.vector.tensor_copy(recs[:], recs_f[:])
                for p in range(2):
                    prf = psa.tile([128, 512], F32, tag="mm")
                    nc.tensor.matmul(prf[:],
                                     sel_s[:, 128 * p:128 * (p + 1)],
                                     recs[:], start=True, stop=True)
                    oidx = 4 * p + ib
                    # even head: normalize into yT8 rows 0:64 directly
                    nc.vector.tensor_mul(
                        yv[0:64, p, 512 * ib:512 * (ib + 1)],
                        ytmp[:, 512 * oidx:512 * (oidx + 1)],
                        prf[0:64, :])
                    # odd head: normalize then DMA into rows 64:128
                    yn = pb.tile([64, 512], BF16, tag="yn", bufs=2,
                                 name="yn")
                    nc.vector.tensor_mul(
                        yn[:],
                        ytmp[:, 4096 + 512 * oidx:4096 + 512 * (oidx + 1)],
                        prf[64:128, :])
                    nc.scalar.dma_start(
                        yv[64:128, p, 512 * ib:512 * (ib + 1)], yn[:])
                # output projection: fp8 DoubleRow, K=256 in one matmul
                for tb in range(4 * ib, 4 * ib + 4):
                    for n in range(2):
                        pj = psa.tile([128, 512], F32, tag="mm")
                        for pp in range(2):
                            nc.tensor.matmul(
                                pj[:], yv[:, pp, 128 * tb:128 * (tb + 1)],
                                wpv[:, pp, 512 * n:512 * (n + 1)],
                                start=(pp == 0), stop=(pp == 1))
                        o = pb.tile([128, 512], BF16, tag="o", bufs=2,
                                    name="o")
                        nc.vector.tensor_copy(o[:], pj[:])
                        nc.sync.dma_start(
                            out_t.ap()[128 * tb:128 * (tb + 1),
                                       512 * n:512 * (n + 1)], o[:])
            return fin

        blocks = [{"p": p, "ib": ib, "jhi": 4 * ib + 3, "pts": [],
                   "py": None, "drained": 0}
                  for ib in range(TB) for p in range(NPAIR)]

        def emit_avs(blk, upto):
            if blk["py"] is None:
                blk["py"] = psy.tile([128, 1024], F32, tag="avy",
                                     name="py")
            py = blk["py"]
            p, jhi = blk["p"], blk["jhi"]
            while blk["drained"] < min(upto, len(blk["pts"])):
                jc, pt, off, w = blk["pts"][blk["drained"]]
                for s in range(2):
                    nc.tensor.matmul(
                        py[0:65, 512 * s + off:512 * (s + 1)],
                        vv[:, 2 * p + s, jc, 0:65],
                        pt[:, 512 * s:512 * s + w],
                        start=(jc == 0), stop=(jc == jhi))
                blk["drained"] += 1

        def finalize(blk):
            py, p, ib = blk["py"], blk["p"], blk["ib"]
            dtmp = pb.tile([1, 1024], F32, tag="dtmp", bufs=2)
            nc.vector.tensor_copy(dtmp[:], py[64:65, :])
            nc.scalar.dma_start(drows[ib][2 * p:2 * p + 1, :],
                                dtmp[:, 0:512])
            nc.scalar.dma_start(drows[ib][2 * p + 1:2 * p + 2, :],
                                dtmp[:, 512:1024])
            oidx = 4 * p + ib
            nc.vector.tensor_copy(
                ytmp[:, 512 * oidx:512 * (oidx + 1)], py[0:64, 0:512])
            nc.vector.tensor_copy(
                ytmp[:, 4096 + 512 * oidx:4096 + 512 * (oidx + 1)],
                py[0:64, 512:1024])

        state = {"pending": None, "prev": None}

        def run_block(blk, achains=(), selfdrain=False):
            p, ib, jhi = blk["p"], blk["ib"], blk["jhi"]
            nst = jhi + 1
            prev = state["prev"]
            nprev = len(prev["pts"]) if prev is not None else 0
            aissued = 0
            for jc in range(nst):
                # drain the previous block's AV chain first: its pt tiles
                # are long since ready, so these stream stall-free while
                # this step's S-pair semaphores settle.
                if prev is not None:
                    emit_avs(prev, (nprev * (jc + 1) + nst - 1) // nst)
                while aissued < (len(achains) * (jc + 1) + nst - 1) // nst:
                    achains[aissued]()
                    aissued += 1
                r = jc - 4 * ib
                off = 128 * r if r > 0 else 0
                w = 512 - off
                ps_pair = pss.tile([128, 1024], F32, tag="spair")
                for s in range(2):
                    nc.tensor.matmul(
                        ps_pair[:, 512 * s:512 * s + w],
                        kTp[p][64 * s:64 * (s + 1),
                               128 * jc:128 * (jc + 1)],
                        qT[p][64 * s:64 * (s + 1),
                              512 * ib + off:512 * (ib + 1)],
                        start=True, stop=True)
                pt = pb.tile([128, 1024], BF16, tag="P", bufs=24,
                             name="pt")
                src = ps_pair[:].rearrange(
                    "p (c w) -> p c w", w=512)[:, :, 0:w]
                dst = pt[:].rearrange(
                    "p (c w) -> p c w", w=512)[:, :, 0:w]
                nc.scalar.activation(dst, src, Exp, scale=0.125)
                if r >= 0:
                    # triangular sub-block = first 128 trimmed cols
                    nc.gpsimd.tensor_mul(
                        pt[:, 0:128], pt[:, 0:128], mask_s[:])
                    nc.gpsimd.tensor_mul(
                        pt[:, 512:640], pt[:, 512:640], mask_s[:])
                blk["pts"].append((jc, pt, off, w))
                if jc == 2 and state["pending"] is not None:
                    state["pending"]()
                    state["pending"] = None
                if selfdrain and jc >= 2:
                    emit_avs(blk, jc - 1)
            if prev is not None:
                emit_avs(prev, nprev)
                finalize(prev)
                if prev["p"] == NPAIR - 1:
                    state["pending"] = make_normproj(prev["ib"])
            state["prev"] = blk

        # ---- interleaved issue: A chains spread into attention steps ----
        for c in phase_a_chains(0):
            c()
        for bi, blk in enumerate(blocks):
            nxt = bi // 2 + 1  # A(nxt) must be fully issued by block 2*nxt
            if nxt < TB:
                half = phase_a_chains(nxt)[0 if bi % 2 == 0 else 4:
                                           4 if bi % 2 == 0 else 8]
            else:
                half = ()
            run_block(blk, achains=half, selfdrain=(bi == len(blocks) - 1))
        prev = state["prev"]
        emit_avs(prev, len(prev["pts"]))
        finalize(prev)
        if state["pending"] is not None:
            state["pending"]()
            state["pending"] = None
        make_normproj(prev["ib"])()

    nc.compile()
    return nc


def _get_compiled():
    global _compiled
    if _compiled is None:
        _compiled = _build_nc()
    return _compiled


def _in_maps(x, w_qkv, w_proj):
    np_bf16 = mybir.dt.np(BF16)
    np_f8 = mybir.dt.np(F8)
    x = np.asarray(x, dtype=np.float32)
    w_qkv = np.asarray(w_qkv, dtype=np.float32)
    w_proj = np.asarray(w_proj, dtype=np.float32)
    mask = _build_mask().astype(np_bf16)
    sel = _build_sel()
    maps = []
    for core in range(N_CORES):
        b = core // 4
        h0 = 4 * (core % 4)
        heads = range(h0, h0 + HPC)
        xt = np.ascontiguousarray(x[b].T)  # [C, T]
        wqk = np.concatenate(
            [w_qkv[:, 64 * h:64 * (h + 1)] for h in heads]
            + [w_qkv[:, C + 64 * h:C + 64 * (h + 1)] for h in heads], axis=1)
        wv = np.concatenate(
            [w_qkv[:, 2 * C + 64 * h:2 * C + 64 * (h + 1)] for h in heads],
            axis=1)
        wp = np.concatenate(
            [w_proj[64 * h:64 * (h + 1), :] for h in heads], axis=0)
        # wp8 [128 d, 2 pair * 1024 c]: row d, pair p -> wp[128p + d, :]
        wp8 = np.ascontiguousarray(
            wp.reshape(2, 128, C).transpose(1, 0, 2).reshape(128, 2 * C))
        maps.append({
            "xt": np.ascontiguousarray(xt).astype(np_bf16),
            "wqk": np.ascontiguousarray(wqk).astype(np_bf16),
            "wv": np.ascontiguousarray(wv).astype(np_bf16),
            "wp": wp8.astype(np_bf16),
            "mask": mask,
            "sel": sel,
            "vinit": np.ones((128, 64 * NJ), dtype=np_bf16),
        })
    return maps


def _combine(results, b_proj):
    out = np.zeros((B, T, C), dtype=np.float32)
    for core in range(N_CORES):
        out[core // 4] += np.asarray(results[core]["out"],
                                     dtype=np.float32)
    out += np.asarray(b_proj, dtype=np.float32)[None, None, :]
    return out


def kernel(x, w_qkv, w_proj, b_proj):
    nc = _get_compiled()
    res = run_bass_kernel_spmd(nc, _in_maps(x, w_qkv, w_proj),
                               core_ids=list(range(N_CORES)))
    return _combine(res.results, b_proj)


def kernel_traced(x, w_qkv, w_proj, b_proj):
    """Like kernel() but with NTFF tracing; returns (out, BassKernelResults)."""
    nc = _get_compiled()
    res = run_bass_kernel_spmd(nc, _in_maps(x, w_qkv, w_proj),
                               core_ids=list(range(N_CORES)), trace=True)
    return _combine(res.results, b_proj), res
